# revision 24
# baseline (speedup 1.0000x reference)
"""EntityAttentionLayer on 8 Trainium2 NeuronCores.

Data-parallel over batch (16 batches/core). All matmuls bf16 with f32 PSUM
accumulation. Layouts avoid PE transposes of activations: q/k are produced
pre-transposed by the projection layout, v naturally, and the attention
output is transposed by the DMA xbar on otherwise-idle DMA queues.

Schedule: one "segment" per batch b interleaves three streams so the PE
never head-of-line blocks on the Scalar exp chain:
  - projections of batch b           (k/v halves; q every 4th batch)
  - attention of batch b-1           (logits pairs between projection
                                      halves drive the exp chain early;
                                      the exp/mask-dependent AV matmuls
                                      are queued late, after enough
                                      projection work to hide the latency)
  - output projection of batch b-2   (attnT was DMA-transposed during
                                      segment b-1)
The K=64 logits matmuls are emitted as row-group pairs (partitions 0-63 /
64-127) so they run concurrently in the PE array. Engine split: Scalar =
8 exps + k/q even-chunk copies + out copy; Vector = 8 mask-muls + the
other PSUM drains + normalize; GpSimd = memset/DMA issue only (its
2-input elemwise path is ~3x slower and steals DVE SBUF ports).
PSUM banks: 4 logits + 2 projection + 2 attention = 8.

Math note: the reference computes
    w = softmax(logits masked with -inf); w[nan] = 0
    w = w * diff; w = w / (sum(w) + 1e-8)
which equals
    num = exp(logits) * valid * diff
    w   = num / (sum(num) + 1e-8 * sum(exp(logits) * valid))
Folding the 1e-8 into the mask: M = valid * (diff + 1e-8) gives
    w ~= exp(logits) * M / sum(exp(logits) * M)
with an O(1e-8) absolute perturbation on w (negligible vs bf16 rounding).
Fully-masked rows: numerator is exactly 0 and the denominator gets +1e-25,
so those rows come out exactly 0, matching the reference's NaN->0 path.
The post-mask and b_out are applied on the host (out returns as bf16).
"""

import numpy as np
import ml_dtypes

BS, NE, NQ = 128, 512, 128
DIN, EMB, ODIM = 512, 512, 512
H, HD = 8, 64
NCORES = 8
BPC = BS // NCORES          # batches per core
GRP = 4                     # batches per q-projection group
EC = DIN // 128             # contraction chunks (4)
BF16 = ml_dtypes.bfloat16


def _build_nc():
    import concourse.bacc as bacc
    import concourse.mybir as mybir
    import concourse.tile as tile
    from concourse.masks import make_identity

    f32 = mybir.dt.float32
    bf16 = mybir.dt.bfloat16

    nc = bacc.Bacc("TRN2", target_bir_lowering=False, debug=False,
                   num_devices=NCORES)

    ents_d = nc.dram_tensor("entsT", [BPC, DIN, NE], bf16, kind="ExternalInput")
    mask_d = nc.dram_tensor("maskT", [BPC, NE, NQ], bf16, kind="ExternalInput")
    win_d = nc.dram_tensor("w_inT", [DIN, 3 * EMB], bf16, kind="ExternalInput")
    wout_d = nc.dram_tensor("w_outT", [EMB, ODIM], bf16, kind="ExternalInput")
    out_d = nc.dram_tensor("out", [BPC, NQ, ODIM], bf16, kind="ExternalOutput")

    with tile.TileContext(nc) as tc:
        with (
            tc.tile_pool(name="const", bufs=1) as cpool,
            tc.tile_pool(name="gwork", bufs=2) as gwork,
            tc.tile_pool(name="work", bufs=4) as work,
            tc.tile_pool(name="nums", bufs=8) as nums,
            tc.tile_pool(name="ps", bufs=2, space="PSUM") as ps,
            tc.tile_pool(name="ps_l", bufs=4, space="PSUM") as ps_l,
            tc.tile_pool(name="ps_att", bufs=2, space="PSUM") as ps_att,
        ):
            # ---- constants (issue order matters: batch 0's k-projection
            # needs only w_in chunk 0 + the first entity slab, so those two
            # DMAs go first and the rest follows the first entity DMA) ----
            ident = cpool.tile([128, 128], bf16)
            make_identity(nc, ident)
            w_in_sb = cpool.tile([128, EC, 3 * EMB], bf16)
            win_r = win_d.ap().rearrange("(c p) f -> p c f", p=128)
            nc.sync.dma_start(out=w_in_sb[:, 0, :], in_=win_r[:, 0, :])
            w_out_sb = cpool.tile([128, EC, ODIM], bf16)

            def late_consts():
                for ce in range(1, EC):
                    nc.sync.dma_start(out=w_in_sb[:, ce, :], in_=win_r[:, ce, :])
                nc.sync.dma_start(
                    out=w_out_sb,
                    in_=wout_d.ap().rearrange("(c p) f -> p c f", p=128))

            # warm-up matmuls: keep the PE HAM busy while the first weight
            # and entity DMAs are in flight so real matmuls start at 2.4 GHz
            psum_w = ps.tile([128, 128], f32, tag="proj", name="psum_w")
            for _ in range(30):
                nc.tensor.matmul(psum_w, lhsT=ident, rhs=ident,
                                 start=True, stop=True)

            def load_group(g):
                ents_sb = gwork.tile([128, EC, GRP, NE], bf16, name="ents_sb")
                for i in range(GRP):
                    nc.sync.dma_start(
                        out=ents_sb[:, :, i, :],
                        in_=ents_d.ap()[g * GRP + i]
                            .rearrange("(c p) n -> p c n", p=128))
                    if g == 0 and i == 0:
                        late_consts()
                return ents_sb

            def load_mask(b):
                mask_sb = work.tile([128, EC, NQ], bf16, name="mask_sb")
                nc.gpsimd.dma_start(
                    out=mask_sb,
                    in_=mask_d.ap()[b].rearrange("(c p) q -> p c q", p=128))
                return mask_sb

            def qproj(ents_sb):
                # fused qT projection for a whole group: qT[f, cf, i, q]
                qT_sb = gwork.tile([128, 4, GRP, NQ], bf16, name="qT_sb")
                for cf in range(4):
                    psum_q = ps.tile([128, GRP, NQ], f32, tag="proj",
                                     name="psum_q")
                    for ce in range(EC):
                        nc.tensor.matmul(
                            psum_q,
                            lhsT=w_in_sb[:, ce, 128 * cf:128 * (cf + 1)],
                            rhs=ents_sb[:, ce, :, 0:NQ],
                            start=(ce == 0), stop=(ce == EC - 1))
                    if cf % 2 == 0:
                        nc.scalar.copy(qT_sb[:, cf, :, :], psum_q)
                    else:
                        nc.vector.tensor_copy(qT_sb[:, cf, :, :], psum_q)
                return qT_sb

            def kproj_half(ents_sb, i, kT_sb, half):
                # kT projection for chunks cf in {2*half, 2*half+1}
                for cf in (2 * half, 2 * half + 1):
                    psum_k = ps.tile([128, NE], f32, tag="proj", name="psum_k")
                    for ce in range(EC):
                        nc.tensor.matmul(
                            psum_k,
                            lhsT=w_in_sb[:, ce,
                                         EMB + 128 * cf:EMB + 128 * (cf + 1)],
                            rhs=ents_sb[:, ce, i, :],
                            start=(ce == 0), stop=(ce == EC - 1))
                    if half == 0:
                        nc.scalar.copy(kT_sb[:, cf, :], psum_k)
                    else:
                        nc.vector.tensor_copy(kT_sb[:, cf, :], psum_k)

            def vproj_half(ents_sb, i, v_sb, half):
                for cn in (2 * half, 2 * half + 1):
                    psum_v = ps.tile([128, EMB], f32, tag="proj", name="psum_v")
                    for ce in range(EC):
                        nc.tensor.matmul(
                            psum_v,
                            lhsT=ents_sb[:, ce, i, 128 * cn:128 * (cn + 1)],
                            rhs=w_in_sb[:, ce, 2 * EMB:3 * EMB],
                            start=(ce == 0), stop=(ce == EC - 1))
                    src = psum_v.rearrange("p (h d) -> p h d", h=H)
                    nc.vector.tensor_copy(v_sb[:, cn, :, 0:HD], src)

            def logits_pair(hc, kT_sb, qT_sb, i):
                # two K=64 matmul streams on disjoint PE row groups (0-63 /
                # 64-127) -> they run concurrently in the array
                psl = [
                    ps_l.tile([128, 4, NQ], f32, tag="logit", name="psl0"),
                    ps_l.tile([128, 4, NQ], f32, tag="logit", name="psl1"),
                ]
                for cn in range(4):
                    for r in range(2):
                        nc.tensor.matmul(
                            psl[r][:, cn, :],
                            lhsT=kT_sb[64 * r:64 * (r + 1), hc,
                                       128 * cn:128 * (cn + 1)],
                            rhs=qT_sb[64 * r:64 * (r + 1), hc, i, :],
                            start=True, stop=True)
                return psl

            def exp_mask(hc, psl, mask_sb, mybir):
                num2 = []
                for r in range(2):
                    h = 2 * hc + r
                    exp_sb = nums.tile([128, 4, NQ], bf16, tag="exp",
                                       name="exp_sb")
                    nc.scalar.activation(
                        exp_sb, psl[r],
                        mybir.ActivationFunctionType.Exp, scale=1.0 / 8.0)
                    num_sb = nums.tile([128, 4, NQ], bf16, tag="num",
                                       name="num_sb")
                    nc.vector.tensor_mul(num_sb, exp_sb, mask_sb)
                    num2.append(num_sb)
                return num2

            def av_pair(hc, num2, v_sb, att_tiles):
                for r in range(2):
                    h = 2 * hc + r
                    patt, j = att_tiles[h // 4], h % 4
                    for cn in range(4):
                        nc.tensor.matmul(
                            patt[:, j, :],
                            lhsT=num2[r][:, cn, :],
                            rhs=v_sb[:, cn, h, :],
                            start=(cn == 0), stop=(cn == 3))

            def tail_v(att_tiles):
                """denominators + normalize (Vector only, no PE work)."""
                dall_sb = work.tile([128, H], f32, name="dall_sb")
                nc.vector.tensor_copy(dall_sb[:, 0:4], att_tiles[0][:, :, HD])
                nc.vector.tensor_copy(dall_sb[:, 4:8], att_tiles[1][:, :, HD])
                deps_sb = work.tile([128, H], f32, name="deps_sb")
                nc.vector.tensor_scalar_add(deps_sb, dall_sb, 1e-25)
                recip_sb = work.tile([128, H], f32, name="recip_sb")
                nc.vector.reciprocal(recip_sb, deps_sb)

                attn_sb = work.tile([128, EMB], bf16, name="attn_sb")
                for h in range(H):
                    nc.vector.tensor_scalar_mul(
                        attn_sb[:, HD * h:HD * (h + 1)],
                        att_tiles[h // 4][:, h % 4, 0:HD],
                        recip_sb[:, h:h + 1])
                return attn_sb

            def transposes(attn_sb, last):
                # transpose attn -> attnT[E, q] on the DMA xbar (keeps the PE
                # queue free of transpose pairs); the final batch spreads
                # across four queues to shorten its fully-exposed chain
                attnT_sb = work.tile([128, 4, 128], bf16, name="attnT_sb")
                engs = ((nc.sync, nc.scalar, nc.sync, nc.scalar) if last
                        else (nc.sync, nc.sync, nc.sync, nc.sync))
                for ct in range(4):
                    engs[ct].dma_start_transpose(
                        attnT_sb[:, ct, :], attn_sb[:, 128 * ct:128 * (ct + 1)])
                return attnT_sb

            def outproj(b, attnT_sb):
                psum_o = ps.tile([128, ODIM], f32, tag="proj", name="psum_o")
                for ct in range(4):
                    nc.tensor.matmul(
                        psum_o,
                        lhsT=attnT_sb[:, ct, :],
                        rhs=w_out_sb[:, ct, :],
                        start=(ct == 0), stop=(ct == 3))
                out_sb = work.tile([128, ODIM], bf16, name="out_sb")
                nc.scalar.copy(out_sb, psum_o)
                nc.gpsimd.dma_start(out=out_d.ap()[b], in_=out_sb)

            import concourse.mybir as mybir_mod

            ents_cur = load_group(0)
            ents_next = None
            qT_cur = None
            # attention context of batch b-1: (kT, v, mask, qT, i)
            actx = None
            # outproj context of batch b-2: attnT_sb
            attnT_prev = None
            attn_pending = None   # attn_sb of batch b-1 (for transposes)

            for b in range(BPC + 2):
                pb = b if b < BPC else None              # projection batch
                ab = b - 1 if 1 <= b <= BPC else None    # attention batch
                ob = b - 2 if b >= 2 else None           # outproj batch

                kT_sb = v_sb = None
                i = pb % GRP if pb is not None else 0
                if pb is not None:
                    if i == 0 and pb > 0:
                        ents_cur, ents_next = ents_next, None
                    kT_sb = work.tile([128, 4, NE], bf16, name="kT_sb")
                    v_sb = work.tile([128, 4, H, HD + 1], bf16, name="v_sb")
                    nc.gpsimd.memset(v_sb[:, :, :, HD], 1.0)

                # kproj chunks 0-1 first (their Scalar copies precede the
                # exps in the queue, so the proj-PSUM rotation never waits
                # behind the exp chain), then the logits pairs drive exp
                if pb is not None:
                    kproj_half(ents_cur, i, kT_sb, 0)
                if ab is not None:
                    a_kT, a_v, a_mask, a_qT, a_i = actx
                    psl0 = logits_pair(0, a_kT, a_qT, a_i)
                    num0 = exp_mask(0, psl0, a_mask, mybir_mod)
                if pb is not None:
                    kproj_half(ents_cur, i, kT_sb, 1)
                if ab is not None:
                    psl1 = logits_pair(1, a_kT, a_qT, a_i)
                    num1 = exp_mask(1, psl1, a_mask, mybir_mod)
                if pb is not None:
                    vproj_half(ents_cur, i, v_sb, 0)
                if ab is not None:
                    psl2 = logits_pair(2, a_kT, a_qT, a_i)
                    num2 = exp_mask(2, psl2, a_mask, mybir_mod)
                    att_tiles = [
                        ps_att.tile([128, 4, HD + 1], f32, tag="att",
                                    name="patt0"),
                        ps_att.tile([128, 4, HD + 1], f32, tag="att",
                                    name="patt1"),
                    ]
                    av_pair(0, num0, a_v, att_tiles)
                if pb is not None:
                    vproj_half(ents_cur, i, v_sb, 1)
                if ab is not None:
                    psl3 = logits_pair(3, a_kT, a_qT, a_i)
                    num3 = exp_mask(3, psl3, a_mask, mybir_mod)
                    av_pair(1, num1, a_v, att_tiles)
                if pb is not None and i == 0:
                    qT_cur = qproj(ents_cur)
                if ab is not None:
                    av_pair(2, num2, a_v, att_tiles)
                if ob is not None:
                    outproj(ob, attnT_prev)
                if ab is not None:
                    av_pair(3, num3, a_v, att_tiles)
                    attn_sb = tail_v(att_tiles)
                    attnT_prev = transposes(attn_sb, last=(ab == BPC - 1))

                if pb is not None:
                    mask_sb = load_mask(pb)
                    if i == 2 and pb + 2 < BPC:
                        ents_next = load_group(pb // GRP + 1)
                    actx = (kT_sb, v_sb, mask_sb, qT_cur, i)

    nc.compile()
    return nc


def _prep_inputs(entities, pre_mask, diff_mask, W_in, W_out):
    entities = np.asarray(entities, dtype=np.float32)
    pre_mask = np.asarray(pre_mask, dtype=bool)
    diff_mask = np.asarray(diff_mask, dtype=np.float32)
    W_in = np.asarray(W_in, dtype=np.float32)
    W_out = np.asarray(W_out, dtype=np.float32)

    entsT = np.ascontiguousarray(entities.transpose(0, 2, 1)).astype(BF16)
    m = (~pre_mask).astype(np.float32) * (diff_mask + 1e-8)
    maskT = np.ascontiguousarray(m.transpose(0, 2, 1)).astype(BF16)
    w_inT = np.ascontiguousarray(W_in.T).astype(BF16)
    w_outT = np.ascontiguousarray(W_out.T).astype(BF16)

    in_maps = []
    for c in range(NCORES):
        sl = slice(c * BPC, (c + 1) * BPC)
        in_maps.append({
            "entsT": np.ascontiguousarray(entsT[sl]),
            "maskT": np.ascontiguousarray(maskT[sl]),
            "w_inT": w_inT,
            "w_outT": w_outT,
        })
    return in_maps


def _run(in_maps, trace=False):
    from concourse.bass_utils import run_bass_kernel_spmd
    nc = _build_nc()
    last_exc = None
    for attempt in range(3):
        try:
            return run_bass_kernel_spmd(
                nc, in_maps, core_ids=list(range(NCORES)), trace=trace)
        except Exception as e:  # transient NRT_EXEC_UNIT faults on fresh NEFFs
            last_exc = e
            import time
            time.sleep(2.0 * (attempt + 1))
    raise last_exc


def kernel_traced(entities, pre_mask, diff_mask, post_mask, W_in, W_out, b_out,
                  trace=False):
    """Returns (output, BassKernelResults)."""
    b_out = np.asarray(b_out, dtype=np.float32)
    post_mask_np = np.asarray(post_mask, dtype=bool)
    in_maps = _prep_inputs(entities, pre_mask, diff_mask, W_in, W_out)
    res = _run(in_maps, trace=trace)
    out = np.concatenate([r["out"] for r in res.results], axis=0)
    out = out.astype(np.float32)
    # post-mask + bias on the host (reference adds b_out before the zeroing)
    out = np.where(post_mask_np[:, :, None], 0.0,
                   out + b_out[None, None, :])
    return out.astype(np.float32), res


def kernel(entities, pre_mask, diff_mask, post_mask, W_in, W_out, b_out):
    out, _ = kernel_traced(entities, pre_mask, diff_mask, post_mask,
                           W_in, W_out, b_out)
    return out
